# revision 6
# baseline (speedup 1.0000x reference)
"""Trainium2 Bass kernel for nn_AutoReg (4-layer dense transformer, teacher forcing).

Sharding across 8 NeuronCores: data-parallel over batch (B=4 -> 4 core pairs),
sequence-split within each pair (causal-balanced interleaved row blocks).
Per-layer K/V are exchanged with one 8-rank AllGather; each core reads its
pair's two segments via partition_id-based dynamic DMA offsets, so the SPMD
program is identical on every core.

Math: bf16 matmul inputs with fp32 PSUM accumulation; LayerNorm, softmax
(exp without max-subtraction; scores are tiny), and the residual stream in
fp32.  Scores are computed transposed (sT[rk, rq]) so the softmax denominator
comes out of the AV matmul via an appended ones-column, and normalization is a
K=1 broadcast matmul + one DVE multiply.
"""

import numpy as np
import ml_dtypes

import concourse.bass as bass
import concourse.bacc as bacc
import concourse.mybir as mybir
import concourse.tile as tile
from concourse.bass import ds
from concourse.bass_utils import run_bass_kernel_spmd
from concourse.masks import make_identity

# Model dims (hardcoded per the problem spec)
L, B, S, D, H, F = 4, 4, 1024, 1024, 16, 4096
V1, V2, OUT = 32, 16, 50
HD = D // H            # 64
NCORES = 8
RLOC = 512             # local rows per core
NRB = RLOC // 128      # 4 local row blocks
NC_ = D // 128         # 8 D-chunks
NFO = F // 128         # 32 F-chunks
SCALE = 1.0 / np.sqrt(HD)

# global row-block assignment per parity (causal-balanced):
# parity 0 (even cores) own blocks [0,1,6,7]; parity 1 own [2,3,4,5]
BLOCKS = {0: [0, 1, 6, 7], 1: [2, 3, 4, 5]}
# global block -> (parity, local block index) - same mapping on every core
G2PL = {0: (0, 0), 1: (0, 1), 6: (0, 2), 7: (0, 3),
        2: (1, 0), 3: (1, 1), 4: (1, 2), 5: (1, 3)}

BF = mybir.dt.bfloat16
F32 = mybir.dt.float32

KSEG = D * RLOC              # 524288 elems: kT region of one core's kv block
VSEG = RLOC * D              # 524288 elems: v region
SEG = KSEG + VSEG            # 1048576 elems per rank in the AllGather


def _build_program():
    nc = bacc.Bacc("TRN2", target_bir_lowering=False)

    # ---- DRAM parameters (per-core inputs) ----
    eat_in = nc.declare_dram_parameter("eat", [64, RLOC], BF, isOutput=False)
    wa_in = nc.declare_dram_parameter("wa", [64, D], BF, isOutput=False)
    pos_in = nc.declare_dram_parameter("pos", [RLOC, D], F32, isOutput=False)
    masks_in = nc.declare_dram_parameter("masks", [128, 8, RLOC], BF, isOutput=False)
    wq_in = nc.declare_dram_parameter("wq", [L * D, D], BF, isOutput=False)
    wk_in = nc.declare_dram_parameter("wk", [L * D, D], BF, isOutput=False)
    wv_in = nc.declare_dram_parameter("wv", [L * D, D], BF, isOutput=False)
    wo_in = nc.declare_dram_parameter("wo", [L * D, D], BF, isOutput=False)
    w1_in = nc.declare_dram_parameter("w1", [L * D, F], BF, isOutput=False)
    w2_in = nc.declare_dram_parameter("w2", [L * F, D], BF, isOutput=False)
    b1_in = nc.declare_dram_parameter("b1", [L * F], F32, isOutput=False)
    b2_in = nc.declare_dram_parameter("b2", [L * D], F32, isOutput=False)
    ln1g_in = nc.declare_dram_parameter("ln1g", [L * D], F32, isOutput=False)
    ln1b_in = nc.declare_dram_parameter("ln1b", [L * D], F32, isOutput=False)
    ln2g_in = nc.declare_dram_parameter("ln2g", [L * D], F32, isOutput=False)
    ln2b_in = nc.declare_dram_parameter("ln2b", [L * D], F32, isOutput=False)
    lnfg_in = nc.declare_dram_parameter("lnfg", [D], F32, isOutput=False)
    lnfb_in = nc.declare_dram_parameter("lnfb", [D], F32, isOutput=False)
    wd_in = nc.declare_dram_parameter("wd", [D, OUT], BF, isOutput=False)
    bd_in = nc.declare_dram_parameter("bd", [OUT], F32, isOutput=False)
    out_p = nc.declare_dram_parameter("out", [RLOC, OUT], F32, isOutput=True)

    def bcast_ap(src_ap, p=128):
        """Partition-broadcast view of a 1-D DRAM AP."""
        return bass.AP(tensor=src_ap.tensor, offset=src_ap.offset,
                       ap=[[0, p]] + [list(x) for x in src_ap.ap])

    AF = mybir.ActivationFunctionType
    ALU = mybir.AluOpType

    with tile.TileContext(nc) as tc:
        with tc.tile_pool(name="res", bufs=1) as res, \
             tc.tile_pool(name="wbig", bufs=2) as wbig, \
             tc.tile_pool(name="yt", bufs=1) as ytp, \
             tc.tile_pool(name="xt", bufs=2) as xtp, \
             tc.tile_pool(name="kvst", bufs=1) as kvst, \
             tc.tile_pool(name="expp", bufs=1) as expp, \
             tc.tile_pool(name="xc", bufs=2) as xcp, \
             tc.tile_pool(name="prm", bufs=2) as prm, \
             tc.tile_pool(name="sm", bufs=6) as sm, \
             tc.tile_pool(name="dr", bufs=1, space="DRAM") as dr, \
             tc.tile_pool(name="ps_big", bufs=2, space="PSUM") as ps_big, \
             tc.tile_pool(name="ps_s", bufs=2, space="PSUM") as ps_s, \
             tc.tile_pool(name="ps_av", bufs=2, space="PSUM") as ps_av, \
             tc.tile_pool(name="ps_bc", bufs=2, space="PSUM") as ps_bc:

            # ---- resident tiles ----
            h_sb = res.tile([128, NRB, D], F32)           # residual stream
            kT_sb = res.tile([128, NC_, S], BF)           # full-seq K^T
            v_sb = res.tile([128, 8, H, HD + 1], BF)      # full-seq V + ones col
            qT_sb = res.tile([128, NC_, RLOC], BF)
            oT_sb = res.tile([128, NC_, RLOC], BF)
            eat_sb = res.tile([64, RLOC], BF)
            wa_sb = res.tile([64, D], BF)
            ident = res.tile([128, 128], F32)
            ones64 = res.tile([1, 64], BF)
            wd_sb = res.tile([128, NC_, OUT], BF)
            bd_bc = res.tile([128, OUT], F32)

            eps_sb = res.tile([128, 1], F32)
            make_identity(nc, ident)
            nc.vector.memset(eps_sb, 1e-6)
            nc.vector.memset(ones64, 1.0)
            nc.vector.memset(v_sb[:, :, :, HD:HD + 1], 1.0)
            nc.sync.dma_start(eat_sb, eat_in[:, :])
            nc.sync.dma_start(wa_sb, wa_in[:, :])
            nc.sync.dma_start(wd_sb, wd_in.rearrange("(c p) n -> p c n", p=128))
            nc.sync.dma_start(bd_bc, bcast_ap(bd_in[:]))

            # dynamic base: rank offset of my pair's even core in kv_all
            pid = nc.sync.partition_id()
            pair_base = (pid // 2) * (2 * SEG)

            # ---- embedding: h = EaT^T @ Wa + pos ----
            pos_sb = wbig.tile([128, NRB, D], F32, tag="w2mb")
            nc.sync.dma_start(pos_sb, pos_in.rearrange("(rb p) d -> p rb d", p=128))
            for rb in range(NRB):
                for o2 in range(2):
                    ps = ps_big.tile([128, 512], F32, tag="big")
                    nc.tensor.matmul(ps, eat_sb[:, 128 * rb:128 * (rb + 1)],
                                     wa_sb[:, 512 * o2:512 * (o2 + 1)],
                                     start=True, stop=True)
                    nc.vector.tensor_add(h_sb[:, rb, 512 * o2:512 * (o2 + 1)],
                                         pos_sb[:, rb, 512 * o2:512 * (o2 + 1)], ps)

            # warm up the ACT exp/ln table set
            warm = sm.tile([128, 1], F32, tag="s1")
            nc.vector.memset(warm, 1.0)
            nc.scalar.activation(warm, warm, AF.Ln, bias=eps_sb, scale=1.0)
            nc.scalar.activation(warm, warm, AF.Exp, bias=0.0, scale=-0.5)

            def layernorm_to_xT(g_src, b_src, xT):
                """LN(h) with affine (g,b), transposed into xT [128, NC_, RLOC] bf16."""
                g_sb = prm.tile([128, NC_], F32, tag="lng")
                b_sb = prm.tile([128, NC_], F32, tag="lnb")
                nc.sync.dma_start(g_sb, g_src.rearrange("(c p) -> p c", p=128))
                nc.sync.dma_start(b_sb, b_src.rearrange("(c p) -> p c", p=128))
                for rb in range(NRB):
                    stats = sm.tile([128, 2, 6], F32, tag="st")
                    nc.vector.bn_stats(stats[:, 0, :], h_sb[:, rb, 0:512])
                    nc.vector.bn_stats(stats[:, 1, :], h_sb[:, rb, 512:1024])
                    mv = sm.tile([128, 2], F32, tag="mv")
                    nc.vector.bn_aggr(mv, stats)
                    rstd = sm.tile([128, 1], F32, tag="rstd")
                    nc.scalar.activation(rstd, mv[:, 1:2], AF.Ln, bias=eps_sb, scale=1.0)
                    nc.scalar.activation(rstd, rstd, AF.Exp, bias=0.0, scale=-0.5)
                    xc = xcp.tile([128, D], F32, tag="xc")
                    nc.vector.tensor_scalar(xc, h_sb[:, rb, :], mv[:, 0:1], rstd,
                                            ALU.subtract, ALU.mult)
                    for c in range(NC_):
                        tp = ps_bc.tile([128, 128], F32, tag="bc")
                        nc.tensor.transpose(tp, xc[:, 128 * c:128 * (c + 1)], ident)
                        nc.vector.tensor_scalar(
                            xT[:, c, 128 * rb:128 * (rb + 1)], tp,
                            g_sb[:, c:c + 1], b_sb[:, c:c + 1], ALU.mult, ALU.add)

            def load_w(src2d, tag="w2mb"):
                w = wbig.tile([128, NC_, src2d.shape[1]], BF, tag=tag)
                nc.sync.dma_start(w, src2d.rearrange("(c p) n -> p c n", p=128))
                return w

            for l in range(L):
                # per-layer kv exchange bounce buffers (Shared tiles allow one writer)
                kv_in = dr.tile([SEG], BF, tag="kvin", name=f"kv_in_{l}")
                kv_all = dr.tile([NCORES * SEG], BF, addr_space="Shared", tag="kvall", name=f"kv_all_{l}")
                # ---- LN1 -> xT ----
                xT = xtp.tile([128, NC_, RLOC], BF, tag="xt")
                layernorm_to_xT(ln1g_in[ds(D * l, D)], ln1b_in[ds(D * l, D)], xT)

                # ---- K projection (own rows) -> bounce ----
                wk_sb = load_w(wk_in[D * l:D * (l + 1)])
                kst = kvst.tile([128, NC_, RLOC], BF, tag="kst")
                for o in range(NC_):
                    ps = ps_big.tile([128, 512], F32, tag="big")
                    for c in range(NC_):
                        nc.tensor.matmul(ps, wk_sb[:, c, 128 * o:128 * (o + 1)],
                                         xT[:, c, :], start=(c == 0), stop=(c == NC_ - 1))
                    nc.vector.tensor_copy(kst[:, o, :], ps)
                nc.sync.dma_start(
                    kv_in[0:KSEG].rearrange("(o p r) -> p o r", o=NC_, p=128), kst)

                # ---- V projection (own rows) -> bounce ----
                wv_sb = load_w(wv_in[D * l:D * (l + 1)])
                vst = kvst.tile([128, NRB, D], BF, tag="vst")
                for rb in range(NRB):
                    for o2 in range(2):
                        ps = ps_big.tile([128, 512], F32, tag="big")
                        for c in range(NC_):
                            nc.tensor.matmul(ps, xT[:, c, 128 * rb:128 * (rb + 1)],
                                             wv_sb[:, c, 512 * o2:512 * (o2 + 1)],
                                             start=(c == 0), stop=(c == NC_ - 1))
                        nc.vector.tensor_copy(vst[:, rb, 512 * o2:512 * (o2 + 1)], ps)
                nc.sync.dma_start(
                    kv_in[KSEG:SEG].rearrange("(rb p d) -> p rb d", rb=NRB, p=128), vst)

                # ---- AllGather K,V across all 8 cores ----
                nc.gpsimd.collective_compute(
                    "AllGather", ALU.bypass,
                    replica_groups=[list(range(NCORES))],
                    ins=[kv_in[:]], outs=[kv_all[:]])

                # ---- Q projection (overlaps the AllGather) ----
                wq_sb = load_w(wq_in[D * l:D * (l + 1)])
                for o in range(NC_):
                    ps = ps_big.tile([128, 512], F32, tag="big")
                    for c in range(NC_):
                        nc.tensor.matmul(ps, wq_sb[:, c, 128 * o:128 * (o + 1)],
                                         xT[:, c, :], start=(c == 0), stop=(c == NC_ - 1))
                    nc.vector.tensor_scalar_mul(qT_sb[:, o, :], ps, float(SCALE))

                # ---- assemble full-sequence kT_sb and v_sb from kv_all ----
                for g in range(8):
                    pg, loc = G2PL[g]
                    roff = pair_base + pg * SEG
                    ksrc = kv_all[ds(roff, KSEG)].rearrange(
                        "(o p r) -> p o r", o=NC_, p=128)
                    nc.sync.dma_start(kT_sb[:, :, 128 * g:128 * (g + 1)],
                                      ksrc[:, :, 128 * loc:128 * (loc + 1)])
                    vsrc = kv_all[ds(roff + KSEG + loc * 128 * D, 128 * D)].rearrange(
                        "(p hh e) -> p hh e", p=128, hh=H)
                    nc.sync.dma_start(v_sb[:, g, :, 0:HD], vsrc)

                # ---- attention, head by head ----
                mask_sb = ytp.tile([128, 8, RLOC], BF, tag="yt")
                nc.sync.dma_start(mask_sb, masks_in[:, :, :])
                for h in range(H):
                    hp, ho = 64 * (h % 2), h // 2
                    expT = expp.tile([128, 8, RLOC], BF, tag="exp")
                    av = ps_av.tile([HD + 1, RLOC], F32, tag="av")
                    for j in range(8):
                        st = ps_s.tile([128, RLOC], F32, tag="s")
                        nc.tensor.matmul(st, kT_sb[hp:hp + 64, ho, 128 * j:128 * (j + 1)],
                                         qT_sb[hp:hp + 64, ho, :], start=True, stop=True)
                        nc.vector.tensor_add(st, st, mask_sb[:, j, :])
                        nc.scalar.activation(expT[:, j, :], st, AF.Exp)
                        nc.tensor.matmul(av, v_sb[:, j, h, :], expT[:, j, :],
                                         start=(j == 0), stop=(j == 7))
                    recip = sm.tile([1, RLOC], BF, tag="recip")
                    with nc.allow_low_precision(reason="softmax denom recip in bf16 is within error budget"):
                        nc.vector.reciprocal(recip, av[HD:HD + 1, :])
                    bc = ps_bc.tile([64, RLOC], F32, tag="bc")
                    nc.tensor.matmul(bc, ones64, recip, start=True, stop=True)
                    bc_sb = sm.tile([64, RLOC], BF, tag="bcsb")
                    nc.vector.tensor_copy(bc_sb, bc)
                    nc.vector.tensor_tensor(oT_sb[hp:hp + 64, ho, :], av[0:HD, :], bc_sb,
                                            ALU.mult)

                # ---- output projection + residual ----
                wo_sb = load_w(wo_in[D * l:D * (l + 1)])
                for rb in range(NRB):
                    for o2 in range(2):
                        ps = ps_big.tile([128, 512], F32, tag="big")
                        for c in range(NC_):
                            nc.tensor.matmul(ps, oT_sb[:, c, 128 * rb:128 * (rb + 1)],
                                             wo_sb[:, c, 512 * o2:512 * (o2 + 1)],
                                             start=(c == 0), stop=(c == NC_ - 1))
                        hsl = h_sb[:, rb, 512 * o2:512 * (o2 + 1)]
                        nc.vector.tensor_add(hsl, hsl, ps)

                # ---- LN2 -> xT2 ----
                xT2 = xtp.tile([128, NC_, RLOC], BF, tag="xt")
                layernorm_to_xT(ln2g_in[ds(D * l, D)], ln2b_in[ds(D * l, D)], xT2)

                # ---- FFN1: yT = relu(w1^T x + b1) ----
                b1_sb = prm.tile([128, NFO], F32, tag="b1")
                nc.sync.dma_start(b1_sb, b1_in[ds(F * l, F)].rearrange("(o p) -> p o", p=128))
                yT = ytp.tile([128, NFO, RLOC], BF, tag="yt")
                for phi in range(4):
                    w1_sb = load_w(w1_in[D * l:D * (l + 1), 1024 * phi:1024 * (phi + 1)])
                    for fo in range(8):
                        fg = 8 * phi + fo
                        ps = ps_big.tile([128, 512], F32, tag="big")
                        for c in range(NC_):
                            nc.tensor.matmul(ps, w1_sb[:, c, 128 * fo:128 * (fo + 1)],
                                             xT2[:, c, :], start=(c == 0), stop=(c == NC_ - 1))
                        nc.vector.tensor_scalar(yT[:, fg, :], ps, b1_sb[:, fg:fg + 1],
                                                0.0, ALU.add, ALU.max)

                # ---- FFN2: h += yT^T @ w2 (+ b2) ----
                for phi in range(4):
                    w2_sb = load_w(w2_in[F * l + 1024 * phi:F * l + 1024 * (phi + 1)])
                    for rb in range(NRB):
                        for o2 in range(2):
                            ps = ps_big.tile([128, 512], F32, tag="big")
                            for c in range(NC_):
                                nc.tensor.matmul(
                                    ps, yT[:, 8 * phi + c, 128 * rb:128 * (rb + 1)],
                                    w2_sb[:, c, 512 * o2:512 * (o2 + 1)],
                                    start=(c == 0), stop=(c == NC_ - 1))
                            hsl = h_sb[:, rb, 512 * o2:512 * (o2 + 1)]
                            nc.vector.tensor_add(hsl, hsl, ps)
                b2_bc = prm.tile([128, D], F32, tag="b2")
                nc.sync.dma_start(b2_bc, bcast_ap(b2_in[ds(D * l, D)]))
                for rb in range(NRB):
                    nc.vector.tensor_add(h_sb[:, rb, :], h_sb[:, rb, :], b2_bc)

            # ---- final LN + decoder ----
            xTf = xtp.tile([128, NC_, RLOC], BF, tag="xt")
            layernorm_to_xT(lnfg_in[:], lnfb_in[:], xTf)
            out_sb = res.tile([128, NRB, OUT], F32)
            for rb in range(NRB):
                ps = ps_bc.tile([128, OUT], F32, tag="bc")
                for c in range(NC_):
                    nc.tensor.matmul(ps, xTf[:, c, 128 * rb:128 * (rb + 1)],
                                     wd_sb[:, c, :], start=(c == 0), stop=(c == NC_ - 1))
                nc.vector.tensor_add(out_sb[:, rb, :], bd_bc, ps)
            nc.sync.dma_start(out_p.rearrange("(rb p) n -> p rb n", p=128), out_sb)

    nc.compile()
    return nc


_PROGRAM = None


def _get_program():
    global _PROGRAM
    if _PROGRAM is None:
        _PROGRAM = _build_program()
    return _PROGRAM


def _bf(x):
    return np.ascontiguousarray(np.asarray(x, np.float32)).astype(ml_dtypes.bfloat16)


def _prep_inputs(inputs):
    """Host-side sharding: build the per-core input maps."""
    I = {k: np.asarray(v) for k, v in inputs.items()}

    wq = _bf(I["wq"].reshape(L * D, D))
    wk = _bf(I["wk"].reshape(L * D, D))
    wv = _bf(I["wv"].reshape(L * D, D))
    wo = _bf(I["wo"].reshape(L * D, D))
    w1 = _bf(I["w1"].reshape(L * D, F))
    w2 = _bf(I["w2"].reshape(L * F, D))
    b1 = np.asarray(I["b1"].reshape(L * F), np.float32)
    b2 = np.asarray(I["b2"].reshape(L * D), np.float32)
    ln1g = np.asarray(I["ln1_g"].reshape(L * D), np.float32)
    ln1b = np.asarray(I["ln1_b"].reshape(L * D), np.float32)
    ln2g = np.asarray(I["ln2_g"].reshape(L * D), np.float32)
    ln2b = np.asarray(I["ln2_b"].reshape(L * D), np.float32)
    lnfg = np.asarray(I["lnf_g"], np.float32)
    lnfb = np.asarray(I["lnf_b"], np.float32)
    wd = _bf(I["wd"])
    bd = np.asarray(I["bd"], np.float32)

    # augmented embedding table [64, D]
    wa = np.zeros((64, D), np.float32)
    wa[0:V1] = I["emb_cat1"]
    wa[V1:V1 + V2] = I["emb_cat2"]
    wa[48] = I["w_num1"][0]
    wa[49] = I["w_num2"][0]
    wa[50] = I["bos"][0, 0]
    wa = _bf(wa)

    pos_emb = np.asarray(I["pos_emb"], np.float32)
    cat1 = np.asarray(I["tgt_cat1"])
    cat2 = np.asarray(I["tgt_cat2"])
    num1 = np.asarray(I["tgt_num1"], np.float32)
    num2 = np.asarray(I["tgt_num2"], np.float32)

    in_maps = []
    shared = dict(wq=wq, wk=wk, wv=wv, wo=wo, w1=w1, w2=w2, b1=b1, b2=b2,
                  ln1g=ln1g, ln1b=ln1b, ln2g=ln2g, ln2b=ln2b,
                  lnfg=lnfg, lnfb=lnfb, wd=wd, bd=bd, wa=wa)
    for c in range(NCORES):
        b, par = c // 2, c % 2
        grows = np.concatenate([np.arange(128 * g, 128 * (g + 1))
                                for g in BLOCKS[par]])          # [512] global rows
        # embedding selector EaT [64, 512]
        eat = np.zeros((64, RLOC), np.float32)
        for r, g in enumerate(grows):
            if g == 0:
                eat[50, r] = 1.0
            else:
                t = g - 1
                eat[cat1[b, t], r] = 1.0
                eat[V1 + cat2[b, t], r] = 1.0
                eat[48, r] = num1[b, t, 0]
                eat[49, r] = num2[b, t, 0]
        # shifted positional embedding [512, D]
        pos = np.zeros((RLOC, D), np.float32)
        nz = grows > 0
        pos[nz] = pos_emb[grows[nz] - 1]
        # additive causal mask [128, 8, 512]
        rk = np.arange(S).reshape(8, 128)                       # [j, p]
        ok = rk.transpose(1, 0)[:, :, None] <= grows[None, None, :]
        mask = np.where(ok, 0.0, -1e9).astype(np.float32)
        in_maps.append(dict(shared,
                            eat=_bf(eat), pos=pos, masks=_bf(mask)))
    return in_maps


def _unshard_output(results):
    out = np.zeros((B, S, OUT), np.float32)
    for c in range(NCORES):
        b, par = c // 2, c % 2
        grows = np.concatenate([np.arange(128 * g, 128 * (g + 1))
                                for g in BLOCKS[par]])
        out[b, grows] = results[c]["out"]
    return out


def kernel(**inputs):
    nc = _get_program()
    in_maps = _prep_inputs(inputs)
    res = run_bass_kernel_spmd(nc, in_maps, core_ids=list(range(NCORES)))
    return _unshard_output(res.results)


def run_traced(inputs):
    """Like kernel() but with NTFF tracing; returns (output, BassKernelResults)."""
    nc = _get_program()
    in_maps = _prep_inputs(inputs)
    res = run_bass_kernel_spmd(nc, in_maps, core_ids=list(range(NCORES)),
                               trace=True, trace_cores=list(range(NCORES)))
    return _unshard_output(res.results), res


# revision 16
# speedup vs baseline: 1.0555x; 1.0555x over previous
"""Trainium2 Bass kernel for nn_AutoReg (4-layer dense transformer, teacher forcing).

Sharding across 8 NeuronCores: data-parallel over batch (B=4 -> 4 core pairs),
sequence-split within each pair (causal-balanced interleaved row blocks).
Per-layer K/V are exchanged with one 8-rank AllGather; each core reads its
pair's two segments via partition_id-based dynamic DMA offsets, so the SPMD
program is identical on every core.

Math: bf16 matmul inputs with fp32 PSUM accumulation; LayerNorm, softmax
(exp without max-subtraction; scores are tiny), and the residual stream in
fp32.  Scores are computed transposed (sT[rk, rq]) so the softmax denominator
comes out of the AV matmul via an appended ones-column, and normalization is a
K=1 broadcast matmul + one DVE multiply.
"""

import numpy as np
import ml_dtypes

import concourse.bass as bass
import concourse.bacc as bacc
import concourse.mybir as mybir
import concourse.tile as tile
from concourse.bass import ds
from concourse.bass_utils import run_bass_kernel_spmd
from concourse.masks import make_identity

# Model dims (hardcoded per the problem spec)
L, B, S, D, H, F = 4, 4, 1024, 1024, 16, 4096
V1, V2, OUT = 32, 16, 50
HD = D // H            # 64
NCORES = 8
RLOC = 512             # local rows per core
NRB = RLOC // 128      # 4 local row blocks
NC_ = D // 128         # 8 D-chunks
NFO = F // 128         # 32 F-chunks
SCALE = 1.0 / np.sqrt(HD)

# global row-block assignment per parity (causal-balanced):
# parity 0 (even cores) own blocks [0,1,6,7]; parity 1 own [2,3,4,5]
BLOCKS = {0: [0, 1, 6, 7], 1: [2, 3, 4, 5]}
# global block -> (parity, local block index) - same mapping on every core
G2PL = {0: (0, 0), 1: (0, 1), 6: (0, 2), 7: (0, 3),
        2: (1, 0), 3: (1, 1), 4: (1, 2), 5: (1, 3)}

BF = mybir.dt.bfloat16
F32 = mybir.dt.float32

KSEG = D * RLOC              # 524288 elems: kT region of one core's kv block
VSEG = RLOC * D              # 524288 elems: v region
SEG = KSEG + VSEG            # 1048576 elems per rank in the AllGather


def _build_program():
    nc = bacc.Bacc("TRN2", target_bir_lowering=False)

    # ---- DRAM parameters (per-core inputs) ----
    eat_in = nc.declare_dram_parameter("eat", [64, RLOC], BF, isOutput=False)
    wa_in = nc.declare_dram_parameter("wa", [64, D], BF, isOutput=False)
    pos_in = nc.declare_dram_parameter("pos", [RLOC, D], F32, isOutput=False)
    masks_in = nc.declare_dram_parameter("masks", [128, 8, RLOC], BF, isOutput=False)
    wq_in = nc.declare_dram_parameter("wq", [L * D, D], BF, isOutput=False)
    wk_in = nc.declare_dram_parameter("wk", [L * D, D], BF, isOutput=False)
    wv_in = nc.declare_dram_parameter("wv", [L * D, D], BF, isOutput=False)
    wo_in = nc.declare_dram_parameter("wo", [L * D, D], BF, isOutput=False)
    w1_in = nc.declare_dram_parameter("w1", [L * D, F], BF, isOutput=False)
    w2_in = nc.declare_dram_parameter("w2", [L * F, D], BF, isOutput=False)
    b1_in = nc.declare_dram_parameter("b1", [L * F], F32, isOutput=False)
    b2_in = nc.declare_dram_parameter("b2", [L * D], F32, isOutput=False)
    ln1g_in = nc.declare_dram_parameter("ln1g", [L * D], F32, isOutput=False)
    ln1b_in = nc.declare_dram_parameter("ln1b", [L * D], F32, isOutput=False)
    ln2g_in = nc.declare_dram_parameter("ln2g", [L * D], F32, isOutput=False)
    ln2b_in = nc.declare_dram_parameter("ln2b", [L * D], F32, isOutput=False)
    lnfg_in = nc.declare_dram_parameter("lnfg", [D], F32, isOutput=False)
    lnfb_in = nc.declare_dram_parameter("lnfb", [D], F32, isOutput=False)
    wd_in = nc.declare_dram_parameter("wd", [D, OUT], BF, isOutput=False)
    bd_in = nc.declare_dram_parameter("bd", [OUT], F32, isOutput=False)
    out_p = nc.declare_dram_parameter("out", [RLOC, OUT], F32, isOutput=True)

    def bcast_ap(src_ap, p=128):
        """Partition-broadcast view of a 1-D DRAM AP."""
        return bass.AP(tensor=src_ap.tensor, offset=src_ap.offset,
                       ap=[[0, p]] + [list(x) for x in src_ap.ap])

    AF = mybir.ActivationFunctionType
    ALU = mybir.AluOpType

    with tile.TileContext(nc) as tc:
        with tc.tile_pool(name="res", bufs=1) as res, \
             tc.tile_pool(name="wbig", bufs=2) as wbig, \
             tc.tile_pool(name="yt", bufs=1) as ytp, \
             tc.tile_pool(name="xt", bufs=2) as xtp, \
             tc.tile_pool(name="kvst", bufs=1) as kvst, \
             tc.tile_pool(name="expp", bufs=1) as expp, \
             tc.tile_pool(name="xc", bufs=2) as xcp, \
             tc.tile_pool(name="prm", bufs=2) as prm, \
             tc.tile_pool(name="sm", bufs=4) as sm, \
             tc.tile_pool(name="dr", bufs=1, space="DRAM") as dr, \
             tc.tile_pool(name="ps_big", bufs=2, space="PSUM") as ps_big, \
             tc.tile_pool(name="ps_s", bufs=2, space="PSUM") as ps_s, \
             tc.tile_pool(name="ps_av", bufs=1, space="PSUM") as ps_av, \
             tc.tile_pool(name="ps_bc", bufs=1, space="PSUM") as ps_bc:

            # ---- resident tiles ----
            h_sb = res.tile([128, NRB, D], F32)           # residual stream
            kT_sb = res.tile([128, NC_, S], BF)           # full-seq K^T
            v_sb = res.tile([128, 8, H, HD + 1], BF)      # full-seq V + ones col
            qT_sb = res.tile([128, NC_, RLOC], BF)
            oT_sb = res.tile([128, NC_, RLOC], BF)
            eat_sb = res.tile([64, RLOC], BF)
            wa_sb = res.tile([64, D], BF)
            ident = res.tile([128, 128], F32)
            ones64 = res.tile([1, 64], BF)
            wd_sb = res.tile([128, NC_, OUT], BF)
            bd_bc = res.tile([128, OUT], F32)

            eps_sb = res.tile([128, 1], F32)
            make_identity(nc, ident)
            nc.vector.memset(eps_sb, 1e-6)
            nc.vector.memset(ones64, 1.0)
            nc.vector.memset(v_sb[:, :, :, HD:HD + 1], 1.0)
            nc.sync.dma_start(eat_sb, eat_in[:, :])
            nc.sync.dma_start(wa_sb, wa_in[:, :])
            nc.sync.dma_start(wd_sb, wd_in.rearrange("(c p) n -> p c n", p=128))
            nc.sync.dma_start(bd_bc, bcast_ap(bd_in[:]))

            # dynamic bases: rank offsets of my pair's even core in k_all / v_all
            pid = nc.sync.partition_id()
            pair_base_k = (pid // 2) * (2 * KSEG)
            pair_base_v = (pid // 2) * (2 * VSEG)

            # ---- embedding: h = EaT^T @ Wa + pos ----
            pos_sb = wbig.tile([128, NRB, D], F32, tag="w2mb")
            nc.sync.dma_start(pos_sb, pos_in.rearrange("(rb p) d -> p rb d", p=128))
            for rb in range(NRB):
                for o2 in range(2):
                    ps = ps_big.tile([128, 512], F32, tag="big")
                    nc.tensor.matmul(ps, eat_sb[:, 128 * rb:128 * (rb + 1)],
                                     wa_sb[:, 512 * o2:512 * (o2 + 1)],
                                     start=True, stop=True)
                    nc.vector.tensor_add(h_sb[:, rb, 512 * o2:512 * (o2 + 1)],
                                         pos_sb[:, rb, 512 * o2:512 * (o2 + 1)], ps)

            # warm up the ACT exp/ln table set
            warm = sm.tile([128, 1], F32, tag="s1")
            nc.vector.memset(warm, 1.0)
            nc.scalar.activation(warm, warm, AF.Ln, bias=eps_sb, scale=1.0)
            nc.scalar.activation(warm, warm, AF.Exp, bias=0.0, scale=-0.5)

            def layernorm_to_xT(g_src, b_src, xT):
                """LN(h) with affine (g,b), transposed into xT [128, NC_, RLOC] bf16."""
                g_sb = prm.tile([128, NC_], F32, tag="lng")
                b_sb = prm.tile([128, NC_], F32, tag="lnb")
                nc.sync.dma_start(g_sb, g_src.rearrange("(c p) -> p c", p=128))
                nc.sync.dma_start(b_sb, b_src.rearrange("(c p) -> p c", p=128))
                for rb in range(NRB):
                    stats = sm.tile([128, 2, 6], F32, tag="st")
                    nc.vector.bn_stats(stats[:, 0, :], h_sb[:, rb, 0:512])
                    nc.vector.bn_stats(stats[:, 1, :], h_sb[:, rb, 512:1024])
                    mv = sm.tile([128, 2], F32, tag="mv")
                    nc.vector.bn_aggr(mv, stats)
                    rstd = sm.tile([128, 1], F32, tag="rstd")
                    nc.scalar.activation(rstd, mv[:, 1:2], AF.Ln, bias=eps_sb, scale=1.0)
                    nc.scalar.activation(rstd, rstd, AF.Exp, bias=0.0, scale=-0.5)
                    xc = xcp.tile([128, D], F32, tag="xc")
                    nc.vector.tensor_scalar(xc, h_sb[:, rb, :], mv[:, 0:1], rstd,
                                            ALU.subtract, ALU.mult)
                    for c in range(NC_):
                        tp = ps_bc.tile([128, 128], F32, tag="bc")
                        nc.tensor.transpose(tp, xc[:, 128 * c:128 * (c + 1)], ident)
                        nc.vector.tensor_scalar(
                            xT[:, c, 128 * rb:128 * (rb + 1)], tp,
                            g_sb[:, c:c + 1], b_sb[:, c:c + 1], ALU.mult, ALU.add)

            def load_w(src2d, tag="w2mb"):
                w = wbig.tile([128, NC_, src2d.shape[1]], BF, tag=tag)
                nc.sync.dma_start(w, src2d.rearrange("(c p) n -> p c n", p=128))
                return w

            for l in range(L):
                # per-layer kv exchange bounce buffers (Shared tiles allow one writer)
                k_in = dr.tile([KSEG], BF, tag="kin", name=f"k_in_{l}")
                v_in = dr.tile([VSEG], BF, tag="vin", name=f"v_in_{l}")
                k_all = dr.tile([NCORES * KSEG], BF, addr_space="Shared", tag="kall", name=f"k_all_{l}")
                v_all = dr.tile([NCORES * VSEG], BF, addr_space="Shared", tag="vall", name=f"v_all_{l}")
                # ---- LN1 -> xT ----
                xT = xtp.tile([128, NC_, RLOC], BF, tag="xt")
                layernorm_to_xT(ln1g_in[ds(D * l, D)], ln1b_in[ds(D * l, D)], xT)

                # ---- K projection (own rows) -> bounce ----
                wk_sb = load_w(wk_in[D * l:D * (l + 1)])
                kst = kvst.tile([128, NC_, RLOC], BF, tag="kst")
                for o in range(NC_):
                    ps = ps_big.tile([128, 512], F32, tag="big")
                    for c in range(NC_):
                        nc.tensor.matmul(ps, wk_sb[:, c, 128 * o:128 * (o + 1)],
                                         xT[:, c, :], start=(c == 0), stop=(c == NC_ - 1))
                    nc.vector.tensor_copy(kst[:, o, :], ps)
                nc.sync.dma_start(
                    k_in.rearrange("(o p r) -> p o r", o=NC_, p=128), kst)
                nc.gpsimd.collective_compute(
                    "AllGather", ALU.bypass,
                    replica_groups=[list(range(NCORES))],
                    ins=[k_in[:]], outs=[k_all[:]])

                # ---- V projection (own rows) -> bounce ----
                wv_sb = load_w(wv_in[D * l:D * (l + 1)])
                vst = kvst.tile([128, NRB, D], BF, tag="vst")
                for rb in range(NRB):
                    for o2 in range(2):
                        ps = ps_big.tile([128, 512], F32, tag="big")
                        for c in range(NC_):
                            nc.tensor.matmul(ps, xT[:, c, 128 * rb:128 * (rb + 1)],
                                             wv_sb[:, c, 512 * o2:512 * (o2 + 1)],
                                             start=(c == 0), stop=(c == NC_ - 1))
                        nc.vector.tensor_copy(vst[:, rb, 512 * o2:512 * (o2 + 1)], ps)
                nc.sync.dma_start(
                    v_in.rearrange("(rb p d) -> p rb d", rb=NRB, p=128), vst)
                nc.gpsimd.collective_compute(
                    "AllGather", ALU.bypass,
                    replica_groups=[list(range(NCORES))],
                    ins=[v_in[:]], outs=[v_all[:]])

                # ---- Q projection (overlaps the AllGather) ----
                wq_sb = load_w(wq_in[D * l:D * (l + 1)])
                for o in range(NC_):
                    ps = ps_big.tile([128, 512], F32, tag="big")
                    for c in range(NC_):
                        nc.tensor.matmul(ps, wq_sb[:, c, 128 * o:128 * (o + 1)],
                                         xT[:, c, :], start=(c == 0), stop=(c == NC_ - 1))
                    nc.vector.tensor_scalar_mul(qT_sb[:, o, :], ps, float(SCALE))

                # ---- assemble full-sequence kT_sb and v_sb ----
                for g in range(8):
                    pg, loc = G2PL[g]
                    ksrc = k_all[ds(pair_base_k + pg * KSEG, KSEG)].rearrange(
                        "(o p r) -> p o r", o=NC_, p=128)
                    nc.sync.dma_start(kT_sb[:, :, 128 * g:128 * (g + 1)],
                                      ksrc[:, :, 128 * loc:128 * (loc + 1)])
                for g in range(8):
                    pg, loc = G2PL[g]
                    vsrc = v_all[ds(pair_base_v + pg * VSEG + loc * 128 * D, 128 * D)].rearrange(
                        "(p hh e) -> p hh e", p=128, hh=H)
                    nc.sync.dma_start(v_sb[:, g, :, 0:HD], vsrc)

                # ---- attention, head by head ----
                mask_sb = ytp.tile([128, 8, RLOC], BF, tag="yt")
                nc.sync.dma_start(mask_sb, masks_in[:, :, :])
                sums_d = dr.tile([H * RLOC], F32, tag="sumsd", name=f"sums_d_{l}")
                recip_d = dr.tile([H * RLOC], BF, tag="recipd", name=f"recip_d_{l}")
                for h in range(H):
                    hp, ho = 64 * (h % 2), h // 2
                    expT = expp.tile([128, 8, RLOC], BF, tag="exp")
                    av = ps_av.tile([HD + 1, RLOC], F32, tag="av")
                    for jp in range(4):
                        st = ps_s.tile([128, 2, RLOC], F32, tag="s")
                        for dj in range(2):
                            j = 2 * jp + dj
                            nc.tensor.matmul(st[:, dj, :],
                                             kT_sb[hp:hp + 64, ho, 128 * j:128 * (j + 1)],
                                             qT_sb[hp:hp + 64, ho, :], start=True, stop=True)
                        nc.scalar.activation(expT[:, 2 * jp:2 * jp + 2, :], st, AF.Exp)
                        nc.vector.tensor_tensor(expT[:, 2 * jp:2 * jp + 2, :],
                                                expT[:, 2 * jp:2 * jp + 2, :],
                                                mask_sb[:, 2 * jp:2 * jp + 2, :], ALU.mult)
                        for dj in range(2):
                            j = 2 * jp + dj
                            nc.tensor.matmul(av, v_sb[:, j, h, :], expT[:, j, :],
                                             start=(j == 0), stop=(j == 7))
                    nc.vector.tensor_copy(oT_sb[hp:hp + 64, ho, :], av[0:HD, :])
                    s1 = sm.tile([1, RLOC], F32, tag="s1h", bufs=2)
                    nc.vector.tensor_copy(s1, av[HD:HD + 1, :])
                    nc.sync.dma_start(
                        sums_d[RLOC * h:RLOC * (h + 1)].rearrange("(a f) -> a f", a=1), s1)
                # reshape through DRAM so the reciprocal runs on 128 partitions
                sums_t = sm.tile([128, H * RLOC // 128], F32, tag="sumt", bufs=1)
                nc.sync.dma_start(sums_t, sums_d.rearrange("(p f) -> p f", p=128))
                recip_t = sm.tile([128, H * RLOC // 128], BF, tag="rect", bufs=1)
                with nc.allow_low_precision(reason="softmax denom recip in bf16 is within error budget"):
                    nc.vector.reciprocal(recip_t, sums_t)
                nc.sync.dma_start(recip_d.rearrange("(p f) -> p f", p=128), recip_t)
                for h in range(H):
                    hp, ho = 64 * (h % 2), h // 2
                    rc1 = sm.tile([1, RLOC], BF, tag="rc1", bufs=2)
                    nc.sync.dma_start(
                        rc1, recip_d[RLOC * h:RLOC * (h + 1)].rearrange("(a f) -> a f", a=1))
                    bc = ps_bc.tile([64, RLOC], F32, tag="bc")
                    nc.tensor.matmul(bc, ones64, rc1, start=True, stop=True)
                    bc_sb = sm.tile([128, RLOC], BF, tag="bcsb", bufs=2)
                    nc.vector.tensor_copy(bc_sb[hp:hp + 64, :], bc)
                    nc.vector.tensor_tensor(oT_sb[hp:hp + 64, ho, :],
                                            oT_sb[hp:hp + 64, ho, :],
                                            bc_sb[hp:hp + 64, :], ALU.mult)

                # ---- output projection + residual ----
                wo_sb = load_w(wo_in[D * l:D * (l + 1)])
                for rb in range(NRB):
                    for o2 in range(2):
                        ps = ps_big.tile([128, 512], F32, tag="big")
                        for c in range(NC_):
                            nc.tensor.matmul(ps, oT_sb[:, c, 128 * rb:128 * (rb + 1)],
                                             wo_sb[:, c, 512 * o2:512 * (o2 + 1)],
                                             start=(c == 0), stop=(c == NC_ - 1))
                        hsl = h_sb[:, rb, 512 * o2:512 * (o2 + 1)]
                        nc.vector.tensor_add(hsl, hsl, ps)

                # ---- LN2 -> xT2 ----
                xT2 = xtp.tile([128, NC_, RLOC], BF, tag="xt")
                layernorm_to_xT(ln2g_in[ds(D * l, D)], ln2b_in[ds(D * l, D)], xT2)

                # ---- FFN1: yT = relu(w1^T x + b1) ----
                b1_sb = prm.tile([128, NFO], F32, tag="b1")
                nc.sync.dma_start(b1_sb, b1_in[ds(F * l, F)].rearrange("(o p) -> p o", p=128))
                yT = ytp.tile([128, NFO, RLOC], BF, tag="yt")
                for phi in range(4):
                    w1_sb = load_w(w1_in[D * l:D * (l + 1), 1024 * phi:1024 * (phi + 1)])
                    for fo in range(8):
                        fg = 8 * phi + fo
                        ps = ps_big.tile([128, 512], F32, tag="big")
                        for c in range(NC_):
                            nc.tensor.matmul(ps, w1_sb[:, c, 128 * fo:128 * (fo + 1)],
                                             xT2[:, c, :], start=(c == 0), stop=(c == NC_ - 1))
                        nc.vector.tensor_scalar(yT[:, fg, :], ps, b1_sb[:, fg:fg + 1],
                                                0.0, ALU.add, ALU.max)

                # ---- FFN2: h += yT^T @ w2 (+ b2) ----
                for phi in range(4):
                    w2_sb = load_w(w2_in[F * l + 1024 * phi:F * l + 1024 * (phi + 1)])
                    for rb in range(NRB):
                        for o2 in range(2):
                            ps = ps_big.tile([128, 512], F32, tag="big")
                            for c in range(NC_):
                                nc.tensor.matmul(
                                    ps, yT[:, 8 * phi + c, 128 * rb:128 * (rb + 1)],
                                    w2_sb[:, c, 512 * o2:512 * (o2 + 1)],
                                    start=(c == 0), stop=(c == NC_ - 1))
                            hsl = h_sb[:, rb, 512 * o2:512 * (o2 + 1)]
                            nc.vector.tensor_add(hsl, hsl, ps)
                b2_bc = prm.tile([128, D], F32, tag="b2")
                nc.sync.dma_start(b2_bc, bcast_ap(b2_in[ds(D * l, D)]))
                for rb in range(NRB):
                    nc.vector.tensor_add(h_sb[:, rb, :], h_sb[:, rb, :], b2_bc)

            # ---- final LN + decoder ----
            xTf = xtp.tile([128, NC_, RLOC], BF, tag="xt")
            layernorm_to_xT(lnfg_in[:], lnfb_in[:], xTf)
            out_sb = res.tile([128, NRB, OUT], F32)
            for rb in range(NRB):
                ps = ps_bc.tile([128, OUT], F32, tag="bc")
                for c in range(NC_):
                    nc.tensor.matmul(ps, xTf[:, c, 128 * rb:128 * (rb + 1)],
                                     wd_sb[:, c, :], start=(c == 0), stop=(c == NC_ - 1))
                nc.vector.tensor_add(out_sb[:, rb, :], bd_bc, ps)
            nc.sync.dma_start(out_p.rearrange("(rb p) n -> p rb n", p=128), out_sb)

    nc.compile()
    return nc


_PROGRAM = None


def _get_program():
    global _PROGRAM
    if _PROGRAM is None:
        _PROGRAM = _build_program()
    return _PROGRAM


def _bf(x):
    return np.ascontiguousarray(np.asarray(x, np.float32)).astype(ml_dtypes.bfloat16)


def _prep_inputs(inputs):
    """Host-side sharding: build the per-core input maps."""
    I = {k: np.asarray(v) for k, v in inputs.items()}

    wq = _bf(I["wq"].reshape(L * D, D))
    wk = _bf(I["wk"].reshape(L * D, D))
    wv = _bf(I["wv"].reshape(L * D, D))
    wo = _bf(I["wo"].reshape(L * D, D))
    w1 = _bf(I["w1"].reshape(L * D, F))
    w2 = _bf(I["w2"].reshape(L * F, D))
    b1 = np.asarray(I["b1"].reshape(L * F), np.float32)
    b2 = np.asarray(I["b2"].reshape(L * D), np.float32)
    ln1g = np.asarray(I["ln1_g"].reshape(L * D), np.float32)
    ln1b = np.asarray(I["ln1_b"].reshape(L * D), np.float32)
    ln2g = np.asarray(I["ln2_g"].reshape(L * D), np.float32)
    ln2b = np.asarray(I["ln2_b"].reshape(L * D), np.float32)
    lnfg = np.asarray(I["lnf_g"], np.float32)
    lnfb = np.asarray(I["lnf_b"], np.float32)
    wd = _bf(I["wd"])
    bd = np.asarray(I["bd"], np.float32)

    # augmented embedding table [64, D]
    wa = np.zeros((64, D), np.float32)
    wa[0:V1] = I["emb_cat1"]
    wa[V1:V1 + V2] = I["emb_cat2"]
    wa[48] = I["w_num1"][0]
    wa[49] = I["w_num2"][0]
    wa[50] = I["bos"][0, 0]
    wa = _bf(wa)

    pos_emb = np.asarray(I["pos_emb"], np.float32)
    cat1 = np.asarray(I["tgt_cat1"])
    cat2 = np.asarray(I["tgt_cat2"])
    num1 = np.asarray(I["tgt_num1"], np.float32)
    num2 = np.asarray(I["tgt_num2"], np.float32)

    in_maps = []
    shared = dict(wq=wq, wk=wk, wv=wv, wo=wo, w1=w1, w2=w2, b1=b1, b2=b2,
                  ln1g=ln1g, ln1b=ln1b, ln2g=ln2g, ln2b=ln2b,
                  lnfg=lnfg, lnfb=lnfb, wd=wd, bd=bd, wa=wa)
    for c in range(NCORES):
        b, par = c // 2, c % 2
        grows = np.concatenate([np.arange(128 * g, 128 * (g + 1))
                                for g in BLOCKS[par]])          # [512] global rows
        # embedding selector EaT [64, 512]
        eat = np.zeros((64, RLOC), np.float32)
        for r, g in enumerate(grows):
            if g == 0:
                eat[50, r] = 1.0
            else:
                t = g - 1
                eat[cat1[b, t], r] = 1.0
                eat[V1 + cat2[b, t], r] = 1.0
                eat[48, r] = num1[b, t, 0]
                eat[49, r] = num2[b, t, 0]
        # shifted positional embedding [512, D]
        pos = np.zeros((RLOC, D), np.float32)
        nz = grows > 0
        pos[nz] = pos_emb[grows[nz] - 1]
        # additive causal mask [128, 8, 512]
        rk = np.arange(S).reshape(8, 128)                       # [j, p]
        ok = rk.transpose(1, 0)[:, :, None] <= grows[None, None, :]
        mask = np.where(ok, 1.0, 0.0).astype(np.float32)
        in_maps.append(dict(shared,
                            eat=_bf(eat), pos=pos, masks=_bf(mask)))
    return in_maps


def _unshard_output(results):
    out = np.zeros((B, S, OUT), np.float32)
    for c in range(NCORES):
        b, par = c // 2, c % 2
        grows = np.concatenate([np.arange(128 * g, 128 * (g + 1))
                                for g in BLOCKS[par]])
        out[b, grows] = results[c]["out"]
    return out


def kernel(**inputs):
    nc = _get_program()
    in_maps = _prep_inputs(inputs)
    res = run_bass_kernel_spmd(nc, in_maps, core_ids=list(range(NCORES)))
    return _unshard_output(res.results)


def run_traced(inputs):
    """Like kernel() but with NTFF tracing; returns (output, BassKernelResults)."""
    nc = _get_program()
    in_maps = _prep_inputs(inputs)
    res = run_bass_kernel_spmd(nc, in_maps, core_ids=list(range(NCORES)),
                               trace=True, trace_cores=list(range(NCORES)))
    return _unshard_output(res.results), res
